# revision 90
# baseline (speedup 1.0000x reference)
"""GPT-2 multi-head causal self-attention on 8 Trainium2 NeuronCores.

Tensor-parallel over heads (2 heads/core), hybrid fp8/bf16 precision:

  - qkv (column-parallel) and c_proj (row-parallel) run in fp8e4m3 with
    DoubleRow perf mode (2 contraction rows per PE cell) for tokens
    512..2047 of each batch; the first 512 tokens -- whose attention rows
    average few keys and so amplify weight-quantization noise -- use a
    bf16 path.  Host pre-interleaves X^T and the weights into the
    [P, 2, N] DoubleRow operand layout, so the fp8 path needs no extra
    on-device data movement.
  - All activations (q/k/v, exp(scores), xa) are stored bf16; PSUM
    accumulation is fp32 throughout.
  - Scores are computed transposed (S^T = K @ Q^T) in [128,2,512] PSUM
    pair-tiles so one exp instruction covers two k-blocks.  Causal
    masking is a gpsimd affine_select restricted to the 128-col diagonal
    band of each diagonal block (plus the pair's garbage columns).
  - V is produced directly in natural [token, channel] layout (X^T
    token-slice stationary, Wv moving) into v_aug tiles with ones
    columns at both edges, so each k-block serves both heads' P@V with
    one 65-wide moving slice and one PSUM drain.
  - P@V runs with the exp-block as the *stationary* operand and natural
    V (plus a ones column for the softmax sum) as the 65-wide moving
    operand, producing x in natural [token, channel] layout where the
    softmax normalization is a per-partition reciprocal+scalar-multiply.
    A PE transpose then returns x^T for the row-parallel c_proj.
  - c_proj partials are written bf16; the host sums the 8 partials in
    float64 and adds the bias (the all-reduce, done host-side).
  - Engine budget per the TimelineSim cost model: PE ~80us, ACT (exp)
    ~93us, DVE ~75us; flexible PSUM drains use nc.any so the Tile
    scheduler load-balances them (gpsimd cannot access PSUM).
"""

import numpy as np
import ml_dtypes

import concourse.bass as bass
import concourse.mybir as mybir
import concourse.tile as tile
from concourse import bacc
from concourse.bass_utils import run_bass_kernel_spmd
from concourse.masks import make_identity

F32 = mybir.dt.float32
BF16 = mybir.dt.bfloat16
F8 = mybir.dt.float8e4
DRM = mybir.MatmulPerfMode.DoubleRow
EXPF = mybir.ActivationFunctionType.Exp
MULT = mybir.AluOpType.mult
ADD = mybir.AluOpType.add

B, S, D, H = 2, 2048, 1024, 16
hd = D // H          # 64
T = B * S            # 4096
NCORES = 8
CW = 2 * hd          # per-core channel width = 128 (2 heads)
KC = 8               # 128-row contraction chunks of D
K2 = 4               # 256-row DoubleRow chunks of D
QC = 4               # 512-wide q superchunks per batch
NKB = S // 128       # 16 k-blocks per batch
SCALE = 1.0 / np.sqrt(hd)
SW = 64.0            # fp8 scale on c_attn weights
SWP = 64.0           # fp8 scale on c_proj weights
SXA = 32.0           # fp8 scale on xa storage
OSC = 1.0 / (SXA * SWP)

_CACHED_NC = None


class _Ring:
    """Manual ring over the second dim of a persistent PSUM tile."""

    def __init__(self, t, n):
        self.t, self.n, self.i = t, n, 0

    def next(self):
        s = self.i % self.n
        self.i += 1
        return s


def _build_nc():
    nc = bacc.Bacc("TRN2", target_bir_lowering=False)
    xbf = nc.dram_tensor("xbf", [128, KC, B, 512], BF16, kind="ExternalInput")
    xf8 = nc.dram_tensor("xf8", [128, K2, 2, B, 3, 512], F8, kind="ExternalInput")
    wbf = nc.dram_tensor("wbf", [128, KC, 3 * CW], BF16, kind="ExternalInput")
    wf8 = nc.dram_tensor("wf8", [128, K2, 2, 3 * CW], F8, kind="ExternalInput")
    bqkv = nc.dram_tensor("bqkv", [3 * CW], F32, kind="ExternalInput")
    wpbf = nc.dram_tensor("wpbf", [64, 2, D], BF16, kind="ExternalInput")
    wpf8 = nc.dram_tensor("wpf8", [64, 2, D], F8, kind="ExternalInput")
    out = nc.dram_tensor("out", [T, D], BF16, kind="ExternalOutput")

    with tile.TileContext(nc) as tc:
        with (
            tc.tile_pool(name="consts", bufs=1) as consts,
            tc.tile_pool(name="big", bufs=1) as big,
            tc.tile_pool(name="ep", bufs=30) as ep,
            tc.tile_pool(name="xnp", bufs=16) as xnp,
            tc.tile_pool(name="rp", bufs=12) as rp,
            tc.tile_pool(name="obp", bufs=8) as obp,
            tc.tile_pool(name="pqp", bufs=2, space="PSUM") as pqp,
            tc.tile_pool(name="psp", bufs=2, space="PSUM") as psp,
            tc.tile_pool(name="pxp", bufs=1, space="PSUM") as pxp,
            tc.tile_pool(name="ptp", bufs=1, space="PSUM") as ptp,
        ):
            # ---- constants ----
            bias_sb = consts.tile([128, 3], F32, tag="b")
            identf = consts.tile([128, 128], F32, tag="idf")
            ident = consts.tile([128, 128], BF16, tag="id")
            w_sb = consts.tile([128, KC, 3 * CW], BF16, tag="w")
            w8_sb = consts.tile([128, K2, 2, 3 * CW], F8, tag="w8")
            wp_sb = consts.tile([64, 2, D], BF16, tag="wp")
            wp8_sb = consts.tile([64, 2, D], F8, tag="wp8")

            # ---- persistent activations ----
            q_T, k_T, v_aug, xb_sb, x8_sb = [], [], [], [], []
            for b in range(B):
                q_T.append(big.tile([128, S], BF16, tag=f"qT{b}", name=f"qT{b}"))
                k_T.append(big.tile([128, S], BF16, tag=f"kT{b}", name=f"kT{b}"))
                # [tok%128, kblock, 132]: [1 1 | V_A | V_B | 1 1]
                v_aug.append(
                    big.tile([128, NKB, 132], BF16, tag=f"va{b}", name=f"va{b}")
                )
                xb_sb.append(
                    big.tile([128, KC, 512], BF16, tag=f"xb{b}", name=f"xb{b}")
                )
                x8_sb.append(
                    big.tile([128, K2, 2, 3, 512], F8, tag=f"x8{b}", name=f"x8{b}")
                )
            # xa, DoubleRow-ready layout [p, head, b, tok]
            xa_e = big.tile([64, 2, B, 512], BF16, tag="xae", name="xa_e")
            xa_dr = big.tile([64, 2, B, 3 * 512], F8, tag="xad", name="xa_dr")

            # first compute needs bias + w(kc 0..) + x(b0, kc 0..)
            nc.sync.dma_start(
                out=bias_sb, in_=bqkv[:].rearrange("(m p) -> p m", p=128)
            )
            nc.sync.dma_start(out=w_sb[:, 0:2], in_=wbf[:, 0:2])
            nc.sync.dma_start(out=xb_sb[0][:, 0:2], in_=xbf[:, 0:2, 0, :])
            nc.sync.dma_start(out=w_sb[:, 2:4], in_=wbf[:, 2:4])
            nc.sync.dma_start(out=xb_sb[0][:, 2:4], in_=xbf[:, 2:4, 0, :])
            nc.sync.dma_start(out=w_sb[:, 4:8], in_=wbf[:, 4:8])
            nc.sync.dma_start(out=xb_sb[0][:, 4:8], in_=xbf[:, 4:8, 0, :])
            nc.sync.dma_start(out=xb_sb[1][:, 0:4], in_=xbf[:, 0:4, 1, :])
            nc.sync.dma_start(out=xb_sb[1][:, 4:8], in_=xbf[:, 4:8, 1, :])
            nc.sync.dma_start(out=w8_sb, in_=wf8[:])
            nc.sync.dma_start(out=x8_sb[0][:, :, :, 0], in_=xf8[:, :, :, 0, 0])
            nc.sync.dma_start(
                out=x8_sb[0][:, :, :, 1], in_=xf8[:, :, :, 0, 1]
            )
            nc.sync.dma_start(
                out=x8_sb[0][:, :, :, 2], in_=xf8[:, :, :, 0, 2]
            )
            make_identity(nc, identf)
            nc.vector.tensor_copy(out=ident, in_=identf)
            for b in range(B):
                nc.vector.memset(v_aug[b][:, :, 0:2], 1.0)
                nc.vector.memset(v_aug[b][:, :, 130:132], 1.0)

            # PSUM rings
            px_t = pxp.tile([128, 7, 65], F32, tag="px", name="px")
            px_ring = _Ring(px_t, 7)
            pr_t = ptp.tile([128, 7, 128], BF16, tag="pr", name="pr")
            pr_ring = _Ring(pr_t, 6)

            # warm-up: keep the PE continuously busy from ~0.3us so the
            # p-state ramp completes before the first real chain
            for wi in range(28):
                nc.tensor.transpose(pr_t[:, 5 + (wi % 2), :], ident, ident)

            _rr = [0]

            def any2():
                _rr[0] ^= 1
                return nc.vector if _rr[0] else nc.gpsimd

            def qkv_store(dst, ps_t, m, fp8):
                # fp8 chains finish in ~430ns: split the psum drain across
                # DVE+Pool so the slot frees at chain rate. bf16 chains are
                # 4x longer; a single copy keeps engine load down.
                if fp8:
                    nc.any.tensor_scalar(
                        out=dst,
                        in0=ps_t,
                        scalar1=1.0 / SW,
                        scalar2=bias_sb[:, m : m + 1],
                        op0=MULT,
                        op1=ADD,
                    )
                else:
                    eng = nc.any
                    eng.tensor_scalar_add(
                        out=dst, in0=ps_t, scalar1=bias_sb[:, m : m + 1]
                    )

            # ---- qkv ----
            def emit_qkv_bf(b, m):
                # bf16 qkv for superchunk 0 of batch b (tokens 0..511)
                ps_t = pqp.tile([128, 512], F32, tag="pq", name="psq")
                for kc in range(KC):
                    nc.tensor.matmul(
                        ps_t,
                        w_sb[:, kc, m * 128 : (m + 1) * 128],
                        xb_sb[b][:, kc],
                        start=(kc == 0),
                        stop=(kc == KC - 1),
                    )
                dst = (q_T[b], k_T[b])[m]
                qkv_store(dst[:, 0:512], ps_t, m, fp8=False)

            def emit_qkv_f8(b, sc, m):
                ps_t = pqp.tile([128, 512], F32, tag="pq", name="psq")
                for k2 in range(K2):
                    nc.tensor.matmul(
                        ps_t,
                        w8_sb[:, k2, :, m * 128 : (m + 1) * 128],
                        x8_sb[b][:, k2, :, sc - 1],
                        start=(k2 == 0),
                        stop=(k2 == K2 - 1),
                        perf_mode=DRM,
                    )
                dst = (q_T[b], k_T[b])[m]
                c0 = sc * 512
                qkv_store(dst[:, c0 : c0 + 512], ps_t, m, fp8=True)

            def emit_vtr(b, blocks):
                # natural-layout V: X^T token-slice stationary, Wv moving.
                # Two 128-token tiles share one pq psum slot so a single
                # wide copy drains both; bias/rescale fold into the copy.
                blocks = list(blocks)
                for c0 in range(0, len(blocks), 2):
                    pair = blocks[c0 : c0 + 2]
                    ptile = pqp.tile([128, 512], F32, tag="pq", name="psq")
                    for u, c in enumerate(pair):
                        pt = ptile[:, u * 128 : (u + 1) * 128]
                        sc, tk = c // 4, (c % 4) * 128
                        if sc == 0:
                            for kc in range(KC):
                                nc.tensor.matmul(
                                    pt,
                                    xb_sb[b][:, kc, tk : tk + 128],
                                    w_sb[:, kc, 256:384],
                                    start=(kc == 0),
                                    stop=(kc == KC - 1),
                                )
                        else:
                            for k2 in range(K2):
                                nc.tensor.matmul(
                                    pt,
                                    x8_sb[b][:, k2, :, sc - 1, tk : tk + 128],
                                    w8_sb[:, k2, :, 256:384],
                                    start=(k2 == 0),
                                    stop=(k2 == K2 - 1),
                                    perf_mode=DRM,
                                )
                    c = pair[0]
                    n = len(pair)
                    sc = c // 4
                    if sc == 0:
                        nc.any.tensor_scalar_add(
                            out=v_aug[b][:, c : c + n, 2:130],
                            in0=ptile[:, 0 : n * 128],
                            scalar1=bias_sb[:, 2:3],
                        )
                    else:
                        nc.any.tensor_scalar(
                            out=v_aug[b][:, c : c + n, 2:130],
                            in0=ptile[:, 0 : n * 128],
                            scalar1=1.0 / SW,
                            scalar2=bias_sb[:, 2:3],
                            op0=MULT,
                            op1=ADD,
                        )

            # ---- attention ----
            def emit_attn(b, h, qi):
                hr = slice(h * 64, (h + 1) * 64)
                vlo = 1 if h == 0 else 66
                scol = 0 if h == 0 else 64
                xlo = 1 if h == 0 else 0
                nkj = 4 * qi + 4
                etiles = []
                for pr in range(nkj // 2):
                    kj0, kj1 = 2 * pr, 2 * pr + 1
                    r0 = kj0 - 4 * qi
                    x0 = 0 if r0 < 2 else 256  # exp'd column range of the pair
                    ps_t = psp.tile([128, 2, 512], F32, tag="ps", name="ps")
                    for i, kj in ((0, kj0), (1, kj1)):
                        r = kj - 4 * qi
                        xs = max(x0, max(0, r) * 128)  # live cols of this block
                        with tc.high_priority(offset=90):
                            nc.tensor.matmul(
                                ps_t[:, i, xs:512],
                                k_T[b][hr, kj * 128 : (kj + 1) * 128],
                                q_T[b][hr, qi * 512 + xs : (qi + 1) * 512],
                                start=True,
                                stop=True,
                            )
                    e_t = ep.tile([128, 2, 512], BF16, tag="e", name="e_t")
                    nc.scalar.activation(
                        out=e_t[:, :, x0:512],
                        in_=ps_t[:, :, x0:512],
                        func=EXPF,
                        scale=float(SCALE),
                    )
                    for i, kj in ((0, kj0), (1, kj1)):
                        r = kj - 4 * qi
                        if r >= 0:
                            # zero garbage + masked cells in [x0, 128r+128)
                            xe = 128 * r + 128
                            nc.gpsimd.affine_select(
                                out=e_t[:, i, x0:xe],
                                in_=e_t[:, i, x0:xe],
                                compare_op=mybir.AluOpType.is_ge,
                                fill=0.0,
                                base=x0 - 128 * r,
                                channel_multiplier=-1,
                                pattern=[[1, xe - x0]],
                            )
                    etiles.append(e_t)
                # P@V per 128-token q-tile, x in natural layout
                for qt in range(4):
                    live = 4 * qi + qt + 1
                    sl = px_ring.next()
                    pxv = px_t[:, sl, :]
                    for kj in range(live):
                        e_t = etiles[kj // 2]
                        nc.tensor.matmul(
                            pxv,
                            e_t[:, kj % 2, qt * 128 : (qt + 1) * 128],
                            v_aug[b][:, kj, vlo : vlo + 65],
                            start=(kj == 0),
                            stop=(kj == live - 1),
                        )
                    rc = rp.tile([128, 1], F32, tag="r", name="rc")
                    with tc.high_priority(offset=100):
                        with nc.allow_low_precision(reason="softmax reciprocal"):
                            nc.vector.reciprocal(
                                out=rc, in_=pxv[:, scol : scol + 1]
                            )
                    xn = xnp.tile([128, 64], BF16, tag="xn", name="xn")
                    nc.any.tensor_scalar_mul(
                        out=xn, in0=pxv[:, xlo : xlo + 64], scalar1=rc
                    )
                    # adjacent pr slots per qt pair; one wide copy drains both
                    sl2 = pr_ring.next()
                    pt = pr_t[0:64, sl2, :]
                    nc.tensor.transpose(pt, xn, ident)
                    if qt % 2 == 1:
                        src2 = pr_t[0:64, sl2 - 1 : sl2 + 1, :]
                        tk = qi * 512 + (qt - 1) * 128
                        if qi == 0:
                            nc.any.tensor_copy(
                                out=xa_e[:, h, b, tk : tk + 256],
                                in_=src2,
                            )
                        else:
                            nc.any.tensor_scalar_mul(
                                out=xa_dr[:, h, b, tk - 512 : tk - 512 + 256],
                                in0=src2,
                                scalar1=SXA,
                            )

            # ---- c_proj ----
            def copy_out(eng, dst, src, scale):
                if eng == "act":
                    if scale == 1.0:
                        nc.scalar.copy(out=dst, in_=src)
                    else:
                        nc.scalar.mul(dst, src, scale)
                elif scale == 1.0:
                    eng.tensor_copy(out=dst, in_=src)
                else:
                    eng.tensor_scalar_mul(out=dst, in0=src, scalar1=scale)

            def emit_C(b, t, engs):
                # one 128-token tile; t<4 bf16 path, else fp8 DoubleRow
                t0 = b * S + t * 128
                ob = obp.tile([128, D], BF16, tag="ob", name="ob")
                for nn in range(2):
                    ps_t = pqp.tile([128, 512], F32, tag="pq", name="psq")
                    dsl = slice(nn * 512, (nn + 1) * 512)
                    if t < 4:
                        for j in range(2):
                            nc.tensor.matmul(
                                ps_t,
                                xa_e[:, j, b, t * 128 : (t + 1) * 128],
                                wp_sb[:, j, dsl],
                                start=(j == 0),
                                stop=(j == 1),
                            )
                        with tc.high_priority(offset=-150):
                            copy_out(engs[nn], ob[:, dsl], ps_t, 1.0)
                    else:
                        tk = (t - 4) * 128
                        nc.tensor.matmul(
                            ps_t,
                            xa_dr[:, :, b, tk : tk + 128],
                            wp8_sb[:, :, dsl],
                            start=True,
                            stop=True,
                            perf_mode=DRM,
                        )
                        with tc.high_priority(offset=-150):
                            copy_out(engs[nn], ob[:, dsl], ps_t, OSC)
                nc.sync.dma_start(out=out[t0 : t0 + 128, :], in_=ob)

            # ---- emission schedule ----
            # Phase A: both batches' bf16 qkv + q0 attention first (fills the
            # ACT pipe early), then b0 ascending with fp8 chains threaded in.
            for m in range(2):
                emit_qkv_bf(0, m)
            emit_vtr(0, range(0, 4))
            emit_attn(0, 0, 0)
            for m in range(2):
                emit_qkv_bf(1, m)
            emit_attn(0, 1, 0)
            emit_vtr(1, range(0, 4))
            for sc in range(3):
                nc.sync.dma_start(
                    out=x8_sb[1][:, :, :, sc], in_=xf8[:, :, :, 1, sc]
                )
            emit_attn(1, 0, 0)
            for m in range(2):
                emit_qkv_f8(0, 1, m)
            emit_attn(1, 1, 0)
            emit_vtr(0, range(4, 8))
            for qi in (1, 2):
                emit_attn(0, 0, qi)
                for m in range(2):
                    emit_qkv_f8(0, qi + 1, m)
                emit_attn(0, 1, qi)
                emit_vtr(0, range(4 * qi + 4, 4 * qi + 8))
            nc.sync.dma_start(out=wp8_sb, in_=wpf8[:])
            nc.sync.dma_start(out=wp_sb, in_=wpbf[:])
            ENGS = [nc.any, nc.any]
            ci = 0

            def next_engs(tail=False):
                nonlocal ci
                ci += 1
                if tail:
                    return ("act", ENGS[ci % 2])
                return (ENGS[ci % 2], ENGS[(ci + 1) % 2])

            emit_attn(0, 0, 3)
            for m in range(2):
                emit_qkv_f8(1, 1, m)
            emit_C(0, 0, next_engs())
            emit_C(0, 1, next_engs())
            emit_attn(0, 1, 3)
            emit_vtr(1, range(4, 8))

            # Phase B: b1 qi 2,3,1 (light tail), remaining c_proj as filler
            c0_pool = list(range(2, 16))

            def pop_c0(k):
                for _ in range(k):
                    if c0_pool:
                        emit_C(0, c0_pool.pop(0), next_engs())

            for m in range(2):
                emit_qkv_f8(1, 2, m)
            emit_vtr(1, range(8, 12))
            emit_attn(1, 0, 1)
            emit_qkv_f8(1, 3, 0)
            pop_c0(1)
            emit_attn(1, 1, 1)
            emit_qkv_f8(1, 3, 1)
            emit_vtr(1, range(12, 16))
            for t in range(0, 4):
                emit_C(1, t, next_engs())
            pop_c0(1)

            emit_attn(1, 0, 2)
            pop_c0(2)
            emit_attn(1, 1, 2)
            for t in range(4, 8):
                emit_C(1, t, next_engs())
            pop_c0(2)

            emit_attn(1, 0, 3)
            pop_c0(3)
            emit_attn(1, 1, 3)
            pop_c0(len(c0_pool))
            for t in range(8, 12):
                emit_C(1, t, next_engs())
            for t in range(12, 16):
                emit_C(1, t, next_engs(tail=True))
    nc.compile()
    return nc


def _get_nc():
    global _CACHED_NC
    if _CACHED_NC is None:
        _CACHED_NC = _build_nc()
    return _CACHED_NC


def _prep_shared(hidden_states):
    """Input-only tensors, identical across cores."""
    F8np = ml_dtypes.float8_e4m3
    BFnp = ml_dtypes.bfloat16
    x = np.asarray(hidden_states, dtype=np.float32).reshape(T, D).T  # [D, T]
    # xbf[p, kc, b, c] = X^T[kc*128+p, b*S + c], c in 0..511
    xbf = np.ascontiguousarray(
        x.reshape(KC, 128, B, QC, 512)[:, :, :, 0, :].transpose(1, 0, 2, 3)
    ).astype(BFnp)
    # xf8[p, k2, j, b, sc-1, c] = X^T[k2*256+j*128+p, b*S + sc*512 + c]
    x6 = x.reshape(K2, 2, 128, B, QC, 512)[:, :, :, :, 1:, :]
    xf8 = np.ascontiguousarray(x6.transpose(2, 0, 1, 3, 4, 5)).astype(F8np)
    return xbf, xf8


def _prep_in_maps(hidden_states, c_attn_w, c_attn_b, c_proj_w):
    F8np = ml_dtypes.float8_e4m3
    BFnp = ml_dtypes.bfloat16
    xbf, xf8 = _prep_shared(hidden_states)
    w = np.asarray(c_attn_w, dtype=np.float32)
    bw = np.asarray(c_attn_b, dtype=np.float32)
    wp = np.asarray(c_proj_w, dtype=np.float32)
    wq, wk, wv = w[:, :D], w[:, D : 2 * D], w[:, 2 * D :]
    bq, bk, bv = bw[:D], bw[D : 2 * D], bw[2 * D :]

    in_maps = []
    for c in range(NCORES):
        cols = slice(c * CW, (c + 1) * CW)
        wl = np.concatenate([wq[:, cols], wk[:, cols], wv[:, cols]], axis=1)
        wbf = np.ascontiguousarray(
            wl.reshape(KC, 128, 3 * CW).transpose(1, 0, 2)
        ).astype(BFnp)
        wf8 = np.ascontiguousarray(
            (wl * SW).reshape(K2, 2, 128, 3 * CW).transpose(2, 0, 1, 3)
        ).astype(F8np)
        b_local = np.concatenate([bq[cols], bk[cols], bv[cols]]).astype(np.float32)
        wpl = wp[cols, :]  # [128, D]
        wpbf = np.ascontiguousarray(wpl.reshape(2, 64, D).transpose(1, 0, 2)).astype(
            BFnp
        )
        wpf8 = np.ascontiguousarray(
            (wpl * SWP).reshape(2, 64, D).transpose(1, 0, 2)
        ).astype(F8np)
        in_maps.append(
            {
                "xbf": xbf,
                "xf8": xf8,
                "wbf": wbf,
                "wf8": wf8,
                "bqkv": np.ascontiguousarray(b_local),
                "wpbf": wpbf,
                "wpf8": wpf8,
            }
        )
    return in_maps


def run_device(hidden_states, c_attn_w, c_attn_b, c_proj_w, c_proj_b, trace=False):
    nc = _get_nc()
    in_maps = _prep_in_maps(hidden_states, c_attn_w, c_attn_b, c_proj_w)
    res = run_bass_kernel_spmd(
        nc, in_maps, core_ids=list(range(NCORES)), trace=trace
    )
    acc = np.zeros((T, D), dtype=np.float64)
    for r in res.results:
        acc += np.asarray(r["out"], dtype=np.float64)
    acc += np.asarray(c_proj_b, dtype=np.float64)[None, :]
    return acc.astype(np.float32).reshape(B, S, D), res


def kernel(hidden_states, c_attn_w, c_attn_b, c_proj_w, c_proj_b):
    out, _ = run_device(hidden_states, c_attn_w, c_attn_b, c_proj_w, c_proj_b)
    return out
